# revision 19
# baseline (speedup 1.0000x reference)
"""Pairwise squared-Euclidean distance matrix kernel for Trainium2.

Computes D[b, i, j] = ||A[b,i] - B[b,j]||^2 for A, B of shape [16, 4096, 256]
fp32, returning [16, 4096, 4096] fp32.

Sharding: data-parallel over the batch dim -- 2 batches per NeuronCore over
8 cores (SPMD: same program, different batch slices).

Per-core algorithm (per batch):
  1. Load B tiles [128, 256] fp32, compute rB = sum(B^2, axis=-1) on ScalarE
     (activation Square + accum_out), PE-transpose each tile into bf16 B^T
     chunk tiles [128(d), 2(k), 512(j)] in SBUF.
  2. Round-trip rB through DRAM to obtain a partition-broadcast copy
     [128, 4096] fp32 in SBUF.
  3. For each 128-row block of A: load, compute rA ([128,1] per-partition),
     PE-transpose, scale by -2 into bf16 (folds the -2 of the cross term).
     For each 512-wide j tile: 2 accumulating bf16 matmuls (k = 256 = 2x128)
     into PSUM, then one VectorE scalar_tensor_tensor:
       out = (psum + rA) + rB_bcast            (fp32)
     After 8 j tiles, DMA the [128, 4096] fp32 row block to the output.

Batch b+1's step-1/2 work is interleaved into batch b's main loop so the
PE/DMA pipelines never drain at batch boundaries.

The bf16 rounding only affects the cross term; |error| ~ 0.1 against
|D| ~ 512, i.e. ~2e-4 relative.
"""

from contextlib import ExitStack

import numpy as np

import concourse.mybir as mybir
import concourse.tile as tile
from concourse import bacc
from concourse.bass import ts
from concourse.masks import make_identity

F32 = mybir.dt.float32
BF16 = mybir.dt.bfloat16

N_CORES = 8
FULL_BATCH = 16
N = 4096
D = 256
P = 128
NT = 512  # output j-tile width (one PSUM bank of fp32)
LOADG = 4  # natural-layout tiles coalesced per input DMA

# j-tiles whose epilogue runs on ScalarE+VectorE instead of VectorE alone
ACT_EPILOG_JTS = ()


def build_nc(b_per_core=FULL_BATCH // N_CORES, n=N, d=D):
    n_itiles = n // P
    n_jtiles = n // NT
    n_ktiles = d // P
    t_per_j = NT // P  # B tiles per bt chunk

    nc = bacc.Bacc()
    a_ext = nc.declare_dram_parameter("A", [b_per_core, n, d], F32, isOutput=False)
    b_ext = nc.declare_dram_parameter("B", [b_per_core, n, d], F32, isOutput=False)
    d_ext = nc.declare_dram_parameter("D", [b_per_core, n, n], F32, isOutput=True)

    with tile.TileContext(nc) as tc, ExitStack() as ctx:
        const_pool = ctx.enter_context(tc.tile_pool(name="const", bufs=1))
        nat_pool = ctx.enter_context(tc.tile_pool(name="nat", bufs=3))
        sq_pool = ctx.enter_context(tc.tile_pool(name="sq", bufs=2))
        bt_pool = ctx.enter_context(tc.tile_pool(name="bt", bufs=2 * n_jtiles))
        at_pool = ctx.enter_context(tc.tile_pool(name="at", bufs=4))
        r_pool = ctx.enter_context(tc.tile_pool(name="r", bufs=2))
        out_pool = ctx.enter_context(tc.tile_pool(name="out", bufs=4))
        psum_mm = ctx.enter_context(tc.tile_pool(name="psum_mm", bufs=3, space="PSUM"))
        psum_tr = ctx.enter_context(tc.tile_pool(name="psum_tr", bufs=2, space="PSUM"))
        dram_pool = ctx.enter_context(tc.tile_pool(name="dram", bufs=2, space="DRAM"))

        ident = const_pool.tile([P, P], F32)
        make_identity(nc, ident)

        # per-batch live state (keyed b % 2 is implied by pool bufs)
        bt_chunks = {}  # (b, jt) -> tile [P, n_ktiles, NT] bf16
        r_b_tiles = {}  # b -> [P, n_itiles] f32
        rb_bcast_tiles = {}  # b -> [P, n] f32

        GW = LOADG * P  # j-width covered by one B group (= NT when LOADG=4)

        def emit_b_group(b, g):
            """Load + process one group of LOADG natural B tiles, including
            this group's slice of the rB broadcast (per-group round trip so
            the first epilogues don't wait on the whole panel)."""
            bn = nat_pool.tile([P, LOADG, d], F32, tag="bn")
            nc.gpsimd.dma_start(
                bn[:],
                b_ext[b, ts(g, LOADG * P), :].rearrange("(t p) d -> p t d", p=P),
            )
            if g == 0:
                rb_bcast_tiles[b] = r_pool.tile(
                    [P, n], F32, tag="rb_bcast", name="rb_bcast"
                )
            r_bg = r_pool.tile([P, LOADG], F32, tag="rbg", name="r_bg")
            for tt in range(LOADG):
                t = g * LOADG + tt
                jt, tj = divmod(t, t_per_j)
                if tj == 0:
                    bt_chunks[(b, jt)] = bt_pool.tile(
                        [P, n_ktiles, NT], BF16, tag="bt", name="bt_chunk"
                    )
                chunk = bt_chunks[(b, jt)]
                sq = sq_pool.tile([P, d], F32, tag="sq")
                nc.scalar.activation(
                    sq[:],
                    bn[:, tt],
                    mybir.ActivationFunctionType.Square,
                    accum_out=r_bg[:, tt : tt + 1],
                )
                for k in range(n_ktiles):
                    ps = psum_tr.tile([P, P], F32, tag="ps_tr")
                    nc.tensor.transpose(ps[:], bn[:, tt, ts(k, P)], ident)
                    nc.scalar.copy(chunk[:, k, ts(tj, P)], ps[:])
            # rB round trip for this group's j-slice (HWDGE only -- keeps
            # the gpsimd Q7 free for SWDGE input-load descriptor generation)
            rb_dram = dram_pool.tile([GW], F32, tag="rb_dram", name="rb_dram")
            nc.sync.dma_start(rb_dram[:].rearrange("(t p) -> p t", p=P), r_bg[:])
            nc.sync.dma_start(
                rb_bcast_tiles[b][:, ts(g, GW)], rb_dram[:].partition_broadcast(P)
            )

        def emit_rb_roundtrip(b):
            pass

        n_bgroups = n_itiles // LOADG  # 8

        def emit_b_pre(b):
            for g in range(n_bgroups):
                emit_b_group(b, g)
            emit_rb_roundtrip(b)

        n_agroups = n_itiles // LOADG

        def load_a_group(b, g):
            t = nat_pool.tile([P, LOADG, d], F32, tag="an", name="an_group")
            nc.gpsimd.dma_start(
                t[:],
                a_ext[b, ts(g, LOADG * P), :].rearrange("(t p) d -> p t d", p=P),
            )
            return t

        an_groups = {0: load_a_group(0, 0)}
        emit_b_pre(0)
        for b in range(b_per_core):
            for it in range(n_itiles):
                # spread next batch's B preprocess across early iterations
                if b + 1 < b_per_core:
                    if it < n_bgroups:
                        emit_b_group(b + 1, it)
                    elif it == n_bgroups:
                        emit_rb_roundtrip(b + 1)

                g, ti = divmod(it, LOADG)
                if ti == 0:
                    # prefetch the next A group one group ahead
                    if g + 1 < n_agroups:
                        an_groups[g + 1] = load_a_group(b, g + 1)
                    elif b + 1 < b_per_core:
                        an_groups[0] = load_a_group(b + 1, 0)
                an = an_groups[g][:, ti]
                r_a = r_pool.tile([P, 1], F32, tag="rA")
                sqa = sq_pool.tile([P, d], F32, tag="sqa")
                nc.scalar.activation(
                    sqa[:],
                    an,
                    mybir.ActivationFunctionType.Square,
                    accum_out=r_a[:],
                )
                at_tile = at_pool.tile([P, n_ktiles, P], BF16, tag="at")
                for k in range(n_ktiles):
                    ps = psum_tr.tile([P, P], F32, tag="ps_tr")
                    nc.tensor.transpose(ps[:], an[:, ts(k, P)], ident)
                    # fold the -2 of "-2*a.b" into the bf16 cast of A^T
                    nc.scalar.mul(at_tile[:, k, :], ps[:], -2.0)

                rb_bcast = rb_bcast_tiles[b]
                out_row = out_pool.tile([P, n], F32, tag="out_row")
                # process j tiles in pairs: one [P, 2*NT] psum tile spans two
                # banks so the epilogue is one DVE op per pair (halves DVE
                # per-op overhead)
                n_jpairs = max(n_jtiles // 2, 1)
                jts_per_pair = n_jtiles // n_jpairs
                for jp in range(n_jpairs):
                    mm_ps = psum_mm.tile([P, jts_per_pair * NT], F32, tag="mm_ps")
                    for jj in range(jts_per_pair):
                        jt = jp * jts_per_pair + jj
                        chunk = bt_chunks[(b, jt)]
                        for k in range(n_ktiles):
                            nc.tensor.matmul(
                                mm_ps[:, ts(jj, NT)],
                                lhsT=at_tile[:, k, :],
                                rhs=chunk[:, k, :],
                                start=(k == 0),
                                stop=(k == n_ktiles - 1),
                            )
                    nc.vector.scalar_tensor_tensor(
                        out=out_row[:, ts(jp, jts_per_pair * NT)],
                        in0=mm_ps[:],
                        scalar=r_a[:],
                        in1=rb_bcast[:, ts(jp, jts_per_pair * NT)],
                        op0=mybir.AluOpType.add,
                        op1=mybir.AluOpType.add,
                    )
                nc.sync.dma_start(d_ext[b, ts(it, P), :], out_row[:])

    nc.compile()
    return nc


_NC_CACHE = {}


def _get_nc(b_per_core, n, d):
    key = (b_per_core, n, d)
    if key not in _NC_CACHE:
        _NC_CACHE[key] = build_nc(b_per_core, n, d)
    return _NC_CACHE[key]


def run(A, B, trace=False, trace_kwargs=None):
    """Run on hardware across 8 cores; returns (D_full, BassKernelResults)."""
    from concourse.bass_utils import run_bass_kernel_spmd

    A = np.ascontiguousarray(np.asarray(A, dtype=np.float32))
    B = np.ascontiguousarray(np.asarray(B, dtype=np.float32))
    full_b = A.shape[0]
    assert full_b % N_CORES == 0
    bpc = full_b // N_CORES
    nc = _get_nc(bpc, A.shape[1], A.shape[2])

    in_maps = [
        {
            "A": A[c * bpc : (c + 1) * bpc],
            "B": B[c * bpc : (c + 1) * bpc],
        }
        for c in range(N_CORES)
    ]
    res = run_bass_kernel_spmd(
        nc,
        in_maps,
        list(range(N_CORES)),
        trace=trace,
        **(trace_kwargs or {}),
    )
    out = np.concatenate([r["D"] for r in res.results], axis=0)
    return out, res


def kernel(A, B):
    out, _ = run(A, B, trace=False)
    return out


# revision 20
# speedup vs baseline: 1.0683x; 1.0683x over previous
"""Pairwise squared-Euclidean distance matrix kernel for Trainium2.

Computes D[b, i, j] = ||A[b,i] - B[b,j]||^2 for A, B of shape [16, 4096, 256]
fp32, returning [16, 4096, 4096] fp32.

Sharding: data-parallel over the batch dim -- 2 batches per NeuronCore over
8 cores (SPMD: same program, different batch slices).

Per-core algorithm (per batch):
  1. Load B tiles [128, 256] fp32, compute rB = sum(B^2, axis=-1) on ScalarE
     (activation Square + accum_out), PE-transpose each tile into bf16 B^T
     chunk tiles [128(d), 2(k), 512(j)] in SBUF.
  2. Round-trip rB through DRAM to obtain a partition-broadcast copy
     [128, 4096] fp32 in SBUF.
  3. For each 128-row block of A: load, compute rA ([128,1] per-partition),
     PE-transpose, scale by -2 into bf16 (folds the -2 of the cross term).
     For each 512-wide j tile: 2 accumulating bf16 matmuls (k = 256 = 2x128)
     into PSUM, then one VectorE scalar_tensor_tensor:
       out = (psum + rA) + rB_bcast            (fp32)
     After 8 j tiles, DMA the [128, 4096] fp32 row block to the output.

Batch b+1's step-1/2 work is interleaved into batch b's main loop so the
PE/DMA pipelines never drain at batch boundaries.

The bf16 rounding only affects the cross term; |error| ~ 0.1 against
|D| ~ 512, i.e. ~2e-4 relative.
"""

from contextlib import ExitStack

import numpy as np

import concourse.mybir as mybir
import concourse.tile as tile
from concourse import bacc
from concourse.bass import ts
from concourse.masks import make_identity

F32 = mybir.dt.float32
BF16 = mybir.dt.bfloat16

N_CORES = 8
FULL_BATCH = 16
N = 4096
D = 256
P = 128
NT = 512  # output j-tile width (one PSUM bank of fp32)
LOADG = 4  # natural-layout tiles coalesced per input DMA

# j-tiles whose epilogue runs on ScalarE+VectorE instead of VectorE alone
ACT_EPILOG_JTS = ()


def build_nc(b_per_core=FULL_BATCH // N_CORES, n=N, d=D):
    n_itiles = n // P
    n_jtiles = n // NT
    n_ktiles = d // P
    t_per_j = NT // P  # B tiles per bt chunk

    nc = bacc.Bacc()
    a_ext = nc.declare_dram_parameter("A", [b_per_core, n, d], F32, isOutput=False)
    b_ext = nc.declare_dram_parameter("B", [b_per_core, n, d], F32, isOutput=False)
    d_ext = nc.declare_dram_parameter("D", [b_per_core, n, n], F32, isOutput=True)

    with tile.TileContext(nc) as tc, ExitStack() as ctx:
        const_pool = ctx.enter_context(tc.tile_pool(name="const", bufs=1))
        nat_pool = ctx.enter_context(tc.tile_pool(name="nat", bufs=3))
        sq_pool = ctx.enter_context(tc.tile_pool(name="sq", bufs=2))
        bt_pool = ctx.enter_context(tc.tile_pool(name="bt", bufs=2 * n_jtiles))
        at_pool = ctx.enter_context(tc.tile_pool(name="at", bufs=4))
        r_pool = ctx.enter_context(tc.tile_pool(name="r", bufs=2))
        rbg_pool = ctx.enter_context(tc.tile_pool(name="rbg", bufs=10))
        ra_pool = ctx.enter_context(tc.tile_pool(name="ra", bufs=6))
        out_pool = ctx.enter_context(tc.tile_pool(name="out", bufs=4))
        psum_mm = ctx.enter_context(tc.tile_pool(name="psum_mm", bufs=3, space="PSUM"))
        psum_tr = ctx.enter_context(tc.tile_pool(name="psum_tr", bufs=2, space="PSUM"))
        dram_pool = ctx.enter_context(tc.tile_pool(name="dram", bufs=2, space="DRAM"))

        ident = const_pool.tile([P, P], F32)
        make_identity(nc, ident)

        # per-batch live state (keyed b % 2 is implied by pool bufs)
        bt_chunks = {}  # (b, jt) -> tile [P, n_ktiles, NT] bf16
        r_b_tiles = {}  # b -> [P, n_itiles] f32
        rb_bcast_tiles = {}  # b -> [P, n] f32

        GW = LOADG * P  # j-width covered by one B group (= NT when LOADG=4)

        def emit_b_group(b, g):
            """Load + process one group of LOADG natural B tiles, including
            this group's slice of the rB broadcast (per-group round trip so
            the first epilogues don't wait on the whole panel)."""
            bn = nat_pool.tile([P, LOADG, d], F32, tag="bn")
            nc.gpsimd.dma_start(
                bn[:],
                b_ext[b, ts(g, LOADG * P), :].rearrange("(t p) d -> p t d", p=P),
            )
            if g == 0:
                rb_bcast_tiles[b] = r_pool.tile(
                    [P, n], F32, tag="rb_bcast", name="rb_bcast"
                )
            r_bg = rbg_pool.tile([P, LOADG], F32, tag="rbg", name="r_bg")
            for tt in range(LOADG):
                t = g * LOADG + tt
                jt, tj = divmod(t, t_per_j)
                if tj == 0:
                    bt_chunks[(b, jt)] = bt_pool.tile(
                        [P, n_ktiles, NT], BF16, tag="bt", name="bt_chunk"
                    )
                chunk = bt_chunks[(b, jt)]
                sq = sq_pool.tile([P, d], F32, tag="sq")
                nc.scalar.activation(
                    sq[:],
                    bn[:, tt],
                    mybir.ActivationFunctionType.Square,
                    accum_out=r_bg[:, tt : tt + 1],
                )
                for k in range(n_ktiles):
                    ps = psum_tr.tile([P, P], F32, tag="ps_tr")
                    nc.tensor.transpose(ps[:], bn[:, tt, ts(k, P)], ident)
                    nc.scalar.copy(chunk[:, k, ts(tj, P)], ps[:])
            # rB round trip for this group's j-slice (HWDGE only -- keeps
            # the gpsimd Q7 free for SWDGE input-load descriptor generation)
            rb_dram = dram_pool.tile([GW], F32, tag="rb_dram", name="rb_dram")
            nc.sync.dma_start(rb_dram[:].rearrange("(t p) -> p t", p=P), r_bg[:])
            nc.sync.dma_start(
                rb_bcast_tiles[b][:, ts(g, GW)], rb_dram[:].partition_broadcast(P)
            )

        def emit_rb_roundtrip(b):
            pass

        n_bgroups = n_itiles // LOADG  # 8

        def emit_b_pre(b):
            for g in range(n_bgroups):
                emit_b_group(b, g)
            emit_rb_roundtrip(b)

        n_agroups = n_itiles // LOADG

        def load_a_group(b, g):
            t = nat_pool.tile([P, LOADG, d], F32, tag="an", name="an_group")
            nc.gpsimd.dma_start(
                t[:],
                a_ext[b, ts(g, LOADG * P), :].rearrange("(t p) d -> p t d", p=P),
            )
            return t

        an_groups = {0: load_a_group(0, 0)}
        emit_b_pre(0)
        for b in range(b_per_core):
            for it in range(n_itiles):
                # spread next batch's B preprocess across early iterations
                if b + 1 < b_per_core:
                    if it < n_bgroups:
                        emit_b_group(b + 1, it)
                    elif it == n_bgroups:
                        emit_rb_roundtrip(b + 1)

                g, ti = divmod(it, LOADG)
                if ti == 0:
                    # prefetch the next A group one group ahead
                    if g + 1 < n_agroups:
                        an_groups[g + 1] = load_a_group(b, g + 1)
                    elif b + 1 < b_per_core:
                        an_groups[0] = load_a_group(b + 1, 0)
                an = an_groups[g][:, ti]
                r_a = ra_pool.tile([P, 1], F32, tag="rA")
                sqa = sq_pool.tile([P, d], F32, tag="sqa")
                nc.scalar.activation(
                    sqa[:],
                    an,
                    mybir.ActivationFunctionType.Square,
                    accum_out=r_a[:],
                )
                at_tile = at_pool.tile([P, n_ktiles, P], BF16, tag="at")
                for k in range(n_ktiles):
                    ps = psum_tr.tile([P, P], F32, tag="ps_tr")
                    nc.tensor.transpose(ps[:], an[:, ts(k, P)], ident)
                    # fold the -2 of "-2*a.b" into the bf16 cast of A^T
                    nc.scalar.mul(at_tile[:, k, :], ps[:], -2.0)

                rb_bcast = rb_bcast_tiles[b]
                out_row = out_pool.tile([P, n], F32, tag="out_row")
                # process j tiles in pairs: one [P, 2*NT] psum tile spans two
                # banks so the epilogue is one DVE op per pair (halves DVE
                # per-op overhead)
                n_jpairs = max(n_jtiles // 2, 1)
                jts_per_pair = n_jtiles // n_jpairs
                for jp in range(n_jpairs):
                    mm_ps = psum_mm.tile([P, jts_per_pair * NT], F32, tag="mm_ps")
                    for jj in range(jts_per_pair):
                        jt = jp * jts_per_pair + jj
                        chunk = bt_chunks[(b, jt)]
                        for k in range(n_ktiles):
                            nc.tensor.matmul(
                                mm_ps[:, ts(jj, NT)],
                                lhsT=at_tile[:, k, :],
                                rhs=chunk[:, k, :],
                                start=(k == 0),
                                stop=(k == n_ktiles - 1),
                            )
                    nc.vector.scalar_tensor_tensor(
                        out=out_row[:, ts(jp, jts_per_pair * NT)],
                        in0=mm_ps[:],
                        scalar=r_a[:],
                        in1=rb_bcast[:, ts(jp, jts_per_pair * NT)],
                        op0=mybir.AluOpType.add,
                        op1=mybir.AluOpType.add,
                    )
                nc.sync.dma_start(d_ext[b, ts(it, P), :], out_row[:])

    nc.compile()
    return nc


_NC_CACHE = {}


def _get_nc(b_per_core, n, d):
    key = (b_per_core, n, d)
    if key not in _NC_CACHE:
        _NC_CACHE[key] = build_nc(b_per_core, n, d)
    return _NC_CACHE[key]


def run(A, B, trace=False, trace_kwargs=None):
    """Run on hardware across 8 cores; returns (D_full, BassKernelResults)."""
    from concourse.bass_utils import run_bass_kernel_spmd

    A = np.ascontiguousarray(np.asarray(A, dtype=np.float32))
    B = np.ascontiguousarray(np.asarray(B, dtype=np.float32))
    full_b = A.shape[0]
    assert full_b % N_CORES == 0
    bpc = full_b // N_CORES
    nc = _get_nc(bpc, A.shape[1], A.shape[2])

    in_maps = [
        {
            "A": A[c * bpc : (c + 1) * bpc],
            "B": B[c * bpc : (c + 1) * bpc],
        }
        for c in range(N_CORES)
    ]
    res = run_bass_kernel_spmd(
        nc,
        in_maps,
        list(range(N_CORES)),
        trace=trace,
        **(trace_kwargs or {}),
    )
    out = np.concatenate([r["D"] for r in res.results], axis=0)
    return out, res


def kernel(A, B):
    out, _ = run(A, B, trace=False)
    return out
